# revision 1
# baseline (speedup 1.0000x reference)
"""MHSA + RoPE kernel for Trainium2, 8 NeuronCores.

Sharding: data-parallel over batch (B=2) x tensor-parallel over heads
(16 heads -> 4 head-groups of 4). Core c handles batch c//4, heads
[4*(c%4) : 4*(c%4)+4]. Each core computes its partial o_proj output
[N, D]; host sums the 4 partials per batch (the "all-reduce").

Device-side layout choices (per core):
  - q/k are computed directly in transposed layout qT/kT [d, n] so the
    scores matmul S^T[j,i] = k_j . q_i needs no transposes, and the PV
    matmul produces A^T [d, i] which is exactly the lhsT layout o_proj
    needs.
  - softmax denominators: S^T tiles are exp'd on ScalarE, accumulated
    elementwise over j-tiles on VectorE, then partition-reduced with a
    ones-vector matmul; reciprocal is broadcast back over partitions
    with a ones-row matmul.
  - projections and o_proj run in bf16 (inputs pre-cast on host);
    scores / PV run in float32r (full-rate fp32 mode of the PE).
"""

import sys

sys.path.insert(0, "/opt/trn_rl_repo")

import numpy as np
import ml_dtypes

import concourse.bass as bass
import concourse.tile as tile
from concourse import bacc, mybir
from concourse.bass_utils import run_bass_kernel_spmd

F32 = mybir.dt.float32
F32R = mybir.dt.float32r
BF16 = mybir.dt.bfloat16
MULT = mybir.AluOpType.mult
ADD = mybir.AluOpType.add
EXP = mybir.ActivationFunctionType.Exp
PSUM = bass.MemorySpace.PSUM

B, N, D = 2, 2048, 2048
H, HD = 16, 128
HL = 4            # local heads per core
C = HL * HD       # 512 local head cols
KT = D // 128     # 16 contraction tiles
NB = 4            # n-blocks of 512 for projections
NT = N // 128     # 16 j-tiles
SCALE = float(HD) ** -0.5
N_CORES = 8

_CACHE = {}


def _build_program():
    nc = bacc.Bacc("TRN2", target_bir_lowering=False, debug=False,
                   num_devices=N_CORES)

    xt_d = nc.dram_tensor("xt", [NB, 128, KT, 512], BF16, kind="ExternalInput")
    wq_d = nc.dram_tensor("wq", [128, KT, C], BF16, kind="ExternalInput")
    wk_d = nc.dram_tensor("wk", [128, KT, C], BF16, kind="ExternalInput")
    wv_d = nc.dram_tensor("wv", [128, KT, C], BF16, kind="ExternalInput")
    wo_d = nc.dram_tensor("wo", [128, HL, D], BF16, kind="ExternalInput")
    cos_d = nc.dram_tensor("cos", [128, N], F32R, kind="ExternalInput")
    sin_d = nc.dram_tensor("sin", [128, N], F32R, kind="ExternalInput")
    onec_d = nc.dram_tensor("onec", [128, 1], F32R, kind="ExternalInput")
    oner_d = nc.dram_tensor("oner", [1, 128], F32R, kind="ExternalInput")
    out_d = nc.dram_tensor("out", [N, D], F32, kind="ExternalOutput")

    with tile.TileContext(nc) as tc:
        with tc.tile_pool(name="res", bufs=1) as res:
            qr = res.tile([128, HL, N], F32R)    # q^T per head [d, n]
            kr = res.tile([128, HL, N], F32R)    # k^T per head [d, n]
            vv = res.tile([128, NT, C], F32R)    # v natural [n, c]
            ones_col = res.tile([128, 1], F32R)
            ones_row = res.tile([1, 128], F32R)
            nc.sync.dma_start(ones_col[:], onec_d[:])
            nc.sync.dma_start(ones_row[:], oner_d[:])

            # ---------------- Phase 1: Q/K/V projections (bf16) ---------
            with (
                tc.tile_pool(name="p1", bufs=1) as p1,
                tc.tile_pool(name="ps1", bufs=6, space=PSUM) as ps1,
            ):
                w_sbs = []
                for wd, wname in ((wq_d, "wq"), (wk_d, "wk"), (wv_d, "wv")):
                    w_sb = p1.tile([128, KT, C], BF16, tag=f"w_{wname}")
                    nc.sync.dma_start(w_sb[:], wd[:])
                    w_sbs.append(w_sb)

                for nb in range(NB):
                    x_sb = p1.tile([128, KT, 512], BF16, tag="x")
                    nc.sync.dma_start(x_sb[:], xt_d[nb])
                    nsl = bass.ts(nb, 512)
                    # q and k projections: psum [d(128) x n(512)] per head
                    for pi, (w_sb, dst) in enumerate(
                        ((w_sbs[0], qr), (w_sbs[1], kr))
                    ):
                        for m in range(HL):
                            ps = ps1.tile([128, 512], F32, tag="ps")
                            for t in range(KT):
                                nc.tensor.matmul(
                                    ps[:],
                                    w_sb[:, t, bass.ts(m, 128)],
                                    x_sb[:, t, :],
                                    start=(t == 0),
                                    stop=(t == KT - 1),
                                )
                            nc.scalar.copy(dst[:, m, nsl], ps[:])
                    # v projection: psum [n(128) x c(512)] per n-chunk
                    for m in range(HL):
                        ps = ps1.tile([128, 512], F32, tag="ps")
                        for t in range(KT):
                            nc.tensor.matmul(
                                ps[:],
                                x_sb[:, t, bass.ts(m, 128)],
                                w_sbs[2][:, t, :],
                                start=(t == 0),
                                stop=(t == KT - 1),
                            )
                        nc.scalar.copy(vv[:, nb * HL + m, :], ps[:])

            # ---------------- Phase 2: RoPE + attention -----------------
            with tc.tile_pool(name="aop", bufs=1) as aop:
                ao = aop.tile([128, HL, N], BF16)   # A^T normalized [c, n]

                with (
                    tc.tile_pool(name="p2", bufs=1) as p2,
                    tc.tile_pool(name="ps_s", bufs=2, space=PSUM) as ps_s,
                    tc.tile_pool(name="ps_a", bufs=1, space=PSUM) as ps_a,
                ):
                    cos_sb = p2.tile([128, N], F32R, tag="cos")
                    sin_sb = p2.tile([128, N], F32R, tag="sin")
                    nc.sync.dma_start(cos_sb[:], cos_d[:])
                    nc.sync.dma_start(sin_sb[:], sin_d[:])

                    # RoPE, in place:  t = shift(q) * sin_signed;
                    # q *= cos; q += t   (sign of sin folded in on host).
                    # The d-half swap is a partition shuffle - compute
                    # engines can't shift partitions, so do it with an
                    # SBUF->SBUF DMA.
                    for src in (qr, kr):
                        for h in range(HL):
                            sl = src[:, h, :]
                            tmp = p2.tile([128, N], F32R, tag="tmp")
                            nc.sync.dma_start(tmp[0:64, :], sl[64:128, :])
                            nc.sync.dma_start(tmp[64:128, :], sl[0:64, :])
                            nc.vector.tensor_tensor(tmp[:], tmp[:], sin_sb[:],
                                                    op=MULT)
                            nc.vector.tensor_tensor(sl, sl, cos_sb[:], op=MULT)
                            nc.vector.tensor_tensor(sl, sl, tmp[:], op=ADD)

                    for h in range(HL):
                        a_ps = ps_a.tile([128, N], F32, tag="a")
                        acc = p2.tile([128, N], F32R, tag="acc")
                        for ih in range(2):
                            ihsl = bass.ts(ih, 1024)
                            for j in range(NT):
                                s_ps = ps_s.tile([128, 1024], F32, tag="s")
                                for f in range(2):
                                    nc.tensor.matmul(
                                        s_ps[:, bass.ts(f, 512)],
                                        kr[:, h, bass.ts(j, 128)],
                                        qr[:, h, ih * 1024 + f * 512 : ih * 1024 + (f + 1) * 512],
                                        start=True, stop=True,
                                    )
                                s_exp = p2.tile([128, 1024], F32R, tag="sexp")
                                nc.scalar.activation(s_exp[:], s_ps[:], EXP,
                                                     scale=SCALE)
                                if j == 0:
                                    nc.vector.tensor_copy(acc[:, ihsl], s_exp[:])
                                else:
                                    nc.vector.tensor_tensor(
                                        acc[:, ihsl], acc[:, ihsl], s_exp[:],
                                        op=ADD)
                                for f in range(2):
                                    nc.tensor.matmul(
                                        a_ps[:, ih * 1024 + f * 512 : ih * 1024 + (f + 1) * 512],
                                        vv[:, j, bass.ts(h, 128)],
                                        s_exp[:, bass.ts(f, 512)],
                                        start=(j == 0), stop=(j == NT - 1),
                                    )
                        # softmax denominators: partition-reduce acc with a
                        # ones matmul, reciprocal, broadcast back over
                        # partitions with a ones-row matmul
                        recip = p2.tile([1, N], F32R, tag="recip")
                        for ih in range(2):
                            l_ps = ps_s.tile([1, 1024], F32, tag="s")
                            for f in range(2):
                                nc.tensor.matmul(
                                    l_ps[:, bass.ts(f, 512)],
                                    ones_col[:],
                                    acc[:, ih * 1024 + f * 512 : ih * 1024 + (f + 1) * 512],
                                    start=True, stop=True,
                                )
                            with nc.allow_low_precision(
                                reason="f32r rounding of softmax recip "
                                       "denominators is ~2^-19 relative"):
                                nc.vector.reciprocal(
                                    recip[:, bass.ts(ih, 1024)], l_ps[:])
                        bc_sb = p2.tile([128, N], F32, tag="bcsb")
                        for ih in range(2):
                            bc_ps = ps_s.tile([128, 1024], F32, tag="s")
                            for f in range(2):
                                nc.tensor.matmul(
                                    bc_ps[:, bass.ts(f, 512)],
                                    ones_row[:],
                                    recip[0:1, ih * 1024 + f * 512 : ih * 1024 + (f + 1) * 512],
                                    start=True, stop=True,
                                )
                            nc.scalar.copy(bc_sb[:, bass.ts(ih, 1024)], bc_ps[:])
                        nc.vector.tensor_tensor(ao[:, h, :], a_ps[:], bc_sb[:],
                                                op=MULT)

                # ---------------- Phase 3: o_proj (bf16) ----------------
                with (
                    tc.tile_pool(name="p3", bufs=1) as p3,
                    tc.tile_pool(name="ps3", bufs=4, space=PSUM) as ps3,
                ):
                    wo_sb = p3.tile([128, HL, D], BF16, tag="wo")
                    nc.sync.dma_start(wo_sb[:], wo_d[:])
                    for m in range(NT):
                        st = p3.tile([128, D], F32, tag="st")
                        for f in range(4):
                            o_ps = ps3.tile([128, 512], F32, tag="o")
                            for ct in range(HL):
                                nc.tensor.matmul(
                                    o_ps[:],
                                    ao[:, ct, bass.ts(m, 128)],
                                    wo_sb[:, ct, bass.ts(f, 512)],
                                    start=(ct == 0), stop=(ct == HL - 1),
                                )
                            nc.scalar.copy(st[:, bass.ts(f, 512)], o_ps[:])
                        nc.sync.dma_start(out_d[bass.ts(m, 128), :], st[:])

    nc.compile()
    return nc


def _rope_tables():
    inv_freq = 1.0 / (10000.0 ** (np.arange(0, HD, 2, dtype=np.float32) / HD))
    pos = np.arange(N, dtype=np.float32)
    freqs = pos[:, None] * inv_freq[None, :]          # [N, HD/2]
    emb = np.concatenate([freqs, freqs], axis=-1)     # [N, HD]
    cos = np.cos(emb).astype(np.float32).T.copy()     # [HD, N]
    sin = np.sin(emb).astype(np.float32).T.copy()     # [HD, N]
    sin_signed = sin.copy()
    sin_signed[0:64] *= -1.0
    return cos, sin_signed


def _make_in_maps(x, Wq, Wk, Wv, Wo):
    cos, sin_signed = _rope_tables()
    bf = ml_dtypes.bfloat16

    in_maps = []
    for c in range(N_CORES):
        b, hg = c // 4, c % 4
        cols = slice(C * hg, C * hg + C)
        xT = np.ascontiguousarray(x[b].T)                      # [D, N]
        xt = np.ascontiguousarray(
            xT.reshape(KT, 128, NB, 512).transpose(2, 1, 0, 3)
        ).astype(bf)                                           # [NB,128,KT,512]

        def wslice(W):
            wt = W[cols, :].T                                  # [D, C]
            return np.ascontiguousarray(
                wt.reshape(KT, 128, C).transpose(1, 0, 2)
            ).astype(bf)                                       # [128, KT, C]

        wo_t = Wo[:, cols].T                                   # [C, D]
        wo = np.ascontiguousarray(
            wo_t.reshape(HL, 128, D).transpose(1, 0, 2)
        ).astype(bf)                                           # [128, HL, D]

        in_maps.append({
            "xt": xt,
            "wq": wslice(Wq),
            "wk": wslice(Wk),
            "wv": wslice(Wv),
            "wo": wo,
            "cos": cos,
            "sin": sin_signed,
            "onec": np.ones((128, 1), dtype=np.float32),
            "oner": np.ones((1, 128), dtype=np.float32),
        })
    return in_maps


def kernel(x, Wq, Wk, Wv, Wo):
    x = np.asarray(x, dtype=np.float32)
    Wq = np.asarray(Wq, dtype=np.float32)
    Wk = np.asarray(Wk, dtype=np.float32)
    Wv = np.asarray(Wv, dtype=np.float32)
    Wo = np.asarray(Wo, dtype=np.float32)

    if "nc" not in _CACHE:
        _CACHE["nc"] = _build_program()
    nc = _CACHE["nc"]

    in_maps = _make_in_maps(x, Wq, Wk, Wv, Wo)
    results = run_bass_kernel_spmd(
        nc, in_maps, core_ids=list(range(N_CORES))
    ).results

    out = np.zeros((B, N, D), dtype=np.float32)
    for c in range(N_CORES):
        out[c // 4] += results[c]["out"]
    return out



# revision 3
# speedup vs baseline: 1.5958x; 1.5958x over previous
"""MHSA + RoPE kernel for Trainium2, 8 NeuronCores.

Sharding: data-parallel over batch (B=2) x tensor-parallel over heads
(16 heads -> 4 head-groups of 4). Core c handles batch c//4, heads
[4*(c%4) : 4*(c%4)+4]. Each core computes its partial o_proj output
[N, D]; host sums the 4 partials per batch (the "all-reduce").

v2 (post-trace): everything on the PE runs bf16 (HW runs f32r matmuls
~2x slower per row than bf16 despite the cost model's parity claim).
Projections run in three passes (k, q, v) with x streamed per pass and
t-outer accumulation so the first matmul starts after one DMA chunk;
RoPE on k overlaps the q pass, RoPE on q overlaps the v pass. The
attention inner loop is split per ih-half with 2-deep PSUM rings and a
3-deep exp ring so scores/exp/PV pipeline across iterations. Softmax
denominators accumulate on VectorE, partition-reduce via a ones-column
matmul, and broadcast back over partitions on GpSimd. Output is bf16.
"""

import sys

sys.path.insert(0, "/opt/trn_rl_repo")

import numpy as np
import ml_dtypes

import concourse.bass as bass
import concourse.tile as tile
from concourse import bacc, mybir
from concourse.bass_utils import run_bass_kernel_spmd

F32 = mybir.dt.float32
F32R = mybir.dt.float32r
BF16 = mybir.dt.bfloat16
MULT = mybir.AluOpType.mult
ADD = mybir.AluOpType.add
EXP = mybir.ActivationFunctionType.Exp
PSUM = bass.MemorySpace.PSUM

B, N, D = 2, 2048, 2048
H, HD = 16, 128
HL = 4            # local heads per core
C = HL * HD       # 512 local head cols
KT = D // 128     # 16 contraction tiles
NB = 4            # n-blocks of 512 for projections
NT = N // 128     # 16 j-tiles
SCALE = float(HD) ** -0.5
N_CORES = 8

_CACHE = {}


def _build_program():
    nc = bacc.Bacc("TRN2", target_bir_lowering=False, debug=False,
                   num_devices=N_CORES)

    xt_d = nc.dram_tensor("xt", [NB, 128, KT, 512], BF16, kind="ExternalInput")
    wq_d = nc.dram_tensor("wq", [128, KT, C], BF16, kind="ExternalInput")
    wk_d = nc.dram_tensor("wk", [128, KT, C], BF16, kind="ExternalInput")
    wv_d = nc.dram_tensor("wv", [128, KT, C], BF16, kind="ExternalInput")
    wo_d = nc.dram_tensor("wo", [128, HL, D], BF16, kind="ExternalInput")
    cos_d = nc.dram_tensor("cos", [128, N], BF16, kind="ExternalInput")
    sin_d = nc.dram_tensor("sin", [128, N], BF16, kind="ExternalInput")
    onec_d = nc.dram_tensor("onec", [128, 1], F32R, kind="ExternalInput")
    out_d = nc.dram_tensor("out", [N, D], BF16, kind="ExternalOutput")

    with tile.TileContext(nc) as tc:
        with tc.tile_pool(name="res", bufs=1) as res:
            qr = res.tile([128, HL, N], BF16)    # q^T per head [d, n]
            kr = res.tile([128, HL, N], BF16)    # k^T per head [d, n]
            vv = res.tile([128, NT, C], BF16)    # v natural [n, c]
            ao = res.tile([128, HL, N], BF16)    # A^T normalized [c, n]
            wo_sb = res.tile([128, HL, D], BF16)
            cos_sb = res.tile([128, N], BF16)
            sin_sb = res.tile([128, N], BF16)
            ones_col = res.tile([128, 1], F32R)
            nc.sync.dma_start(ones_col[:], onec_d[:])
            nc.sync.dma_start(cos_sb[:], cos_d[:])
            nc.sync.dma_start(sin_sb[:], sin_d[:])

            # ---------------- Phase 1: Q/K/V projections (bf16) ---------
            with (
                tc.tile_pool(name="p1", bufs=1) as p1,
                tc.tile_pool(name="ps1", bufs=2, space=PSUM) as ps1,
            ):
                w_sbs = {}
                for wd, wname in ((wk_d, "wk"), (wq_d, "wq"), (wv_d, "wv")):
                    w_sb = p1.tile([128, KT, C], BF16, tag=f"w_{wname}")
                    for tch in range(4):
                        tsl = slice(4 * tch, 4 * tch + 4)
                        nc.sync.dma_start(w_sb[:, tsl, :], wd[:, tsl, :])
                    w_sbs[wname] = w_sb

                def proj_pass(wname, dst, kind, extra=None):
                    w_sb = w_sbs[wname]
                    for nb in range(NB):
                        x_sb = p1.tile([128, KT, 512], BF16, tag="x", bufs=2)
                        for tch in range(4):
                            tsl = slice(4 * tch, 4 * tch + 4)
                            nc.sync.dma_start(x_sb[:, tsl, :],
                                              xt_d[nb][:, tsl, :])
                        pss = [ps1.tile([128, 512], F32, tag=f"pp{m}",
                                        name=f"pp{m}")
                               for m in range(HL)]
                        for t in range(KT):
                            for m in range(HL):
                                if kind == "qk":
                                    lhsT = w_sb[:, t, bass.ts(m, 128)]
                                    rhs = x_sb[:, t, :]
                                else:
                                    lhsT = x_sb[:, t, bass.ts(m, 128)]
                                    rhs = w_sb[:, t, :]
                                nc.tensor.matmul(
                                    pss[m], lhsT, rhs,
                                    start=(t == 0), stop=(t == KT - 1),
                                )
                        for m in range(HL):
                            if kind == "qk":
                                nc.scalar.copy(dst[:, m, bass.ts(nb, 512)],
                                               pss[m])
                            else:
                                nc.vector.tensor_copy(dst[:, nb * HL + m, :],
                                                      pss[m])
                        if extra is not None and nb == 0:
                            extra()

                def rope(src):
                    # t = shift(src) * sin_signed; src *= cos; src += t
                    # (sign of sin folded in on host). The d-half swap is a
                    # partition shuffle - done with SBUF->SBUF DMA.
                    for h in range(HL):
                        sl = src[:, h, :]
                        tmp = p1.tile([128, N], BF16, tag="tmp", bufs=2)
                        nc.sync.dma_start(tmp[0:64, :], sl[64:128, :])
                        nc.sync.dma_start(tmp[64:128, :], sl[0:64, :])
                        nc.vector.tensor_tensor(tmp[:], tmp[:], sin_sb[:],
                                                op=MULT)
                        nc.vector.tensor_tensor(sl, sl, cos_sb[:], op=MULT)
                        nc.vector.tensor_tensor(sl, sl, tmp[:], op=ADD)

                def load_wo():
                    for hch in range(HL):
                        nc.sync.dma_start(wo_sb[:, hch, :], wo_d[:, hch, :])

                proj_pass("wk", kr, "qk")
                rope(kr)          # DVE + DMA, overlaps q pass on PE
                proj_pass("wq", qr, "qk")
                rope(qr)          # overlaps v pass on PE
                proj_pass("wv", vv, "v", extra=load_wo)

            # ---------------- Phase 2: RoPE'd attention (bf16) ----------
            with (
                tc.tile_pool(name="p2", bufs=1) as p2,
                tc.tile_pool(name="ps_s", bufs=2, space=PSUM) as ps_s,
                tc.tile_pool(name="ps_a", bufs=2, space=PSUM) as ps_a,
            ):
                for h in range(HL):
                    for ih in range(2):
                        ihsl = bass.ts(ih, 1024)
                        a_ps = ps_a.tile([128, 1024], F32, tag="a")
                        acc = p2.tile([128, 1024], F32R, tag="acc", bufs=2)
                        for j in range(NT):
                            s_ps = ps_s.tile([128, 1024], F32, tag="s")
                            for f in range(2):
                                q0 = ih * 1024 + f * 512
                                nc.tensor.matmul(
                                    s_ps[:, bass.ts(f, 512)],
                                    kr[:, h, bass.ts(j, 128)],
                                    qr[:, h, q0:q0 + 512],
                                    start=True, stop=True,
                                )
                            s_exp = p2.tile([128, 1024], BF16, tag="sexp",
                                            bufs=3)
                            nc.scalar.activation(s_exp[:], s_ps[:], EXP,
                                                 scale=SCALE)
                            if j == 0:
                                nc.vector.tensor_copy(acc[:], s_exp[:])
                            else:
                                nc.vector.tensor_tensor(acc[:], acc[:],
                                                        s_exp[:], op=ADD)
                            for f in range(2):
                                nc.tensor.matmul(
                                    a_ps[:, bass.ts(f, 512)],
                                    vv[:, j, bass.ts(h, 128)],
                                    s_exp[:, bass.ts(f, 512)],
                                    start=(j == 0), stop=(j == NT - 1),
                                )
                        # softmax denominators: partition-reduce acc with a
                        # ones-column matmul, reciprocal on VectorE, then
                        # broadcast over partitions on GpSimd.
                        l_ps = ps_s.tile([1, 1024], F32, tag="s")
                        for f in range(2):
                            nc.tensor.matmul(
                                l_ps[:, bass.ts(f, 512)],
                                ones_col[:],
                                acc[:, bass.ts(f, 512)],
                                start=True, stop=True,
                            )
                        recip = p2.tile([1, 1024], F32, tag="recip", bufs=2)
                        nc.vector.reciprocal(recip[:], l_ps[:])
                        bc_sb = p2.tile([128, 1024], F32, tag="bcsb", bufs=2)
                        nc.gpsimd.partition_broadcast(bc_sb[:], recip[0:1, :])
                        nc.vector.tensor_tensor(ao[:, h, ihsl], a_ps[:],
                                                bc_sb[:], op=MULT)

            # ---------------- Phase 3: o_proj (bf16) --------------------
            with (
                tc.tile_pool(name="p3", bufs=1) as p3,
                tc.tile_pool(name="ps3", bufs=2, space=PSUM) as ps3,
            ):
                for m in range(NT):
                    st = p3.tile([128, D], BF16, tag="st", bufs=2)
                    for f in range(4):
                        o_ps = ps3.tile([128, 512], F32, tag=f"o{f % 2}")
                        for ct in range(HL):
                            nc.tensor.matmul(
                                o_ps[:],
                                ao[:, ct, bass.ts(m, 128)],
                                wo_sb[:, ct, bass.ts(f, 512)],
                                start=(ct == 0), stop=(ct == HL - 1),
                            )
                        if f < 2:
                            nc.scalar.copy(st[:, bass.ts(f, 512)], o_ps[:])
                        else:
                            nc.vector.tensor_copy(st[:, bass.ts(f, 512)],
                                                  o_ps[:])
                    nc.sync.dma_start(out_d[bass.ts(m, 128), :], st[:])

    nc.compile()
    return nc


def _rope_tables():
    inv_freq = 1.0 / (10000.0 ** (np.arange(0, HD, 2, dtype=np.float32) / HD))
    pos = np.arange(N, dtype=np.float32)
    freqs = pos[:, None] * inv_freq[None, :]          # [N, HD/2]
    emb = np.concatenate([freqs, freqs], axis=-1)     # [N, HD]
    cos = np.cos(emb).astype(np.float32).T.copy()     # [HD, N]
    sin = np.sin(emb).astype(np.float32).T.copy()     # [HD, N]
    sin_signed = sin.copy()
    sin_signed[0:64] *= -1.0
    return cos, sin_signed


def _make_in_maps(x, Wq, Wk, Wv, Wo):
    cos, sin_signed = _rope_tables()
    bf = ml_dtypes.bfloat16

    in_maps = []
    for c in range(N_CORES):
        b, hg = c // 4, c % 4
        cols = slice(C * hg, C * hg + C)
        xT = np.ascontiguousarray(x[b].T)                      # [D, N]
        xt = np.ascontiguousarray(
            xT.reshape(KT, 128, NB, 512).transpose(2, 1, 0, 3)
        ).astype(bf)                                           # [NB,128,KT,512]

        def wslice(W):
            wt = W[cols, :].T                                  # [D, C]
            return np.ascontiguousarray(
                wt.reshape(KT, 128, C).transpose(1, 0, 2)
            ).astype(bf)                                       # [128, KT, C]

        wo_t = Wo[:, cols].T                                   # [C, D]
        wo = np.ascontiguousarray(
            wo_t.reshape(HL, 128, D).transpose(1, 0, 2)
        ).astype(bf)                                           # [128, HL, D]

        in_maps.append({
            "xt": xt,
            "wq": wslice(Wq),
            "wk": wslice(Wk),
            "wv": wslice(Wv),
            "wo": wo,
            "cos": cos.astype(bf),
            "sin": sin_signed.astype(bf),
            "onec": np.ones((128, 1), dtype=np.float32),
        })
    return in_maps


def kernel(x, Wq, Wk, Wv, Wo):
    x = np.asarray(x, dtype=np.float32)
    Wq = np.asarray(Wq, dtype=np.float32)
    Wk = np.asarray(Wk, dtype=np.float32)
    Wv = np.asarray(Wv, dtype=np.float32)
    Wo = np.asarray(Wo, dtype=np.float32)

    if "nc" not in _CACHE:
        _CACHE["nc"] = _build_program()
    nc = _CACHE["nc"]

    in_maps = _make_in_maps(x, Wq, Wk, Wv, Wo)
    results = run_bass_kernel_spmd(
        nc, in_maps, core_ids=list(range(N_CORES))
    ).results

    out = np.zeros((B, N, D), dtype=np.float32)
    for c in range(N_CORES):
        out[c // 4] += np.asarray(results[c]["out"], dtype=np.float32)
    return out


# revision 8
# speedup vs baseline: 1.8858x; 1.1818x over previous
"""MHSA + RoPE kernel for Trainium2, 8 NeuronCores.

Sharding: data-parallel over batch (B=2) x tensor-parallel over heads
(16 heads -> 4 head-groups of 4). Core c handles batch c//4, heads
[4*(c%4) : 4*(c%4)+4]. Each core computes its partial o_proj output
[N, D]; host sums the 4 partials per batch (the "all-reduce").

v2 (post-trace): everything on the PE runs bf16 (HW runs f32r matmuls
~2x slower per row than bf16 despite the cost model's parity claim).
Projections run in three passes (k, q, v) with x streamed per pass and
t-outer accumulation so the first matmul starts after one DMA chunk;
RoPE on k overlaps the q pass, RoPE on q overlaps the v pass. The
attention inner loop is split per ih-half with 2-deep PSUM rings and a
3-deep exp ring so scores/exp/PV pipeline across iterations. Softmax
denominators accumulate on VectorE, partition-reduce via a ones-column
matmul, and broadcast back over partitions on GpSimd. Output is bf16.
"""

import sys

sys.path.insert(0, "/opt/trn_rl_repo")

import numpy as np
import ml_dtypes

import concourse.bass as bass
import concourse.tile as tile
from concourse import bacc, mybir
from concourse.bass_utils import run_bass_kernel_spmd

F32 = mybir.dt.float32
F32R = mybir.dt.float32r
BF16 = mybir.dt.bfloat16
MULT = mybir.AluOpType.mult
ADD = mybir.AluOpType.add
EXP = mybir.ActivationFunctionType.Exp
PSUM = bass.MemorySpace.PSUM

B, N, D = 2, 2048, 2048
H, HD = 16, 128
HL = 4            # local heads per core
C = HL * HD       # 512 local head cols
KT = D // 128     # 16 contraction tiles
NB = 4            # n-blocks of 512 for projections
NT = N // 128     # 16 j-tiles
SCALE = float(HD) ** -0.5
N_CORES = 8

_CACHE = {}


def _build_program():
    nc = bacc.Bacc("TRN2", target_bir_lowering=False, debug=False,
                   num_devices=N_CORES)

    xt_d = nc.dram_tensor("xt", [NB, 128, KT, 512], BF16, kind="ExternalInput")
    wq_d = nc.dram_tensor("wq", [128, KT, C], BF16, kind="ExternalInput")
    wk_d = nc.dram_tensor("wk", [128, KT, C], BF16, kind="ExternalInput")
    wv_d = nc.dram_tensor("wv", [128, KT, C], BF16, kind="ExternalInput")
    wo_d = nc.dram_tensor("wo", [128, HL, D], BF16, kind="ExternalInput")
    cos_d = nc.dram_tensor("cos", [128, N], BF16, kind="ExternalInput")
    sin_d = nc.dram_tensor("sin", [128, N], BF16, kind="ExternalInput")
    onec_d = nc.dram_tensor("onec", [128, 1], BF16, kind="ExternalInput")
    out_d = nc.dram_tensor("out", [N, D], BF16, kind="ExternalOutput")

    with tile.TileContext(nc) as tc:
        with tc.tile_pool(name="res", bufs=1) as res:
            qr = res.tile([128, HL, N], BF16)    # q^T per head [d, n]
            kr = res.tile([128, HL, N], BF16)    # k^T per head [d, n]
            vv = res.tile([128, NT, C], BF16)    # v natural [n, c]
            ao = res.tile([128, HL, N], BF16)    # A^T normalized [c, n]
            wo_sb = res.tile([128, HL, D], BF16)
            cos_sb = res.tile([128, N], BF16)
            sin_sb = res.tile([128, N], BF16)
            ones_col = res.tile([128, 1], BF16)
            nc.sync.dma_start(ones_col[:], onec_d[:])
            nc.sync.dma_start(cos_sb[:], cos_d[:])
            nc.sync.dma_start(sin_sb[:], sin_d[:])

            # ---------------- Phase 1: Q/K/V projections (bf16) ---------
            # q+k in one pass over x (8 psum chains), then v in a second
            # pass. x chunks stream on the SP DMA queue; weights + RoPE
            # swaps ride the Activation HWDGE queue so the two overlap.
            with (
                tc.tile_pool(name="p1", bufs=1) as p1,
                tc.tile_pool(name="ps1", bufs=1, space=PSUM) as ps1,
            ):
                w_sbs = {}
                for wd, wname in ((wk_d, "wk"), (wq_d, "wq"), (wv_d, "wv")):
                    w_sb = p1.tile([128, KT, C], BF16, tag=f"w_{wname}")
                    nch = 4 if wname == "wk" else 2
                    for tch in range(nch):
                        tsl = slice(KT // nch * tch, KT // nch * (tch + 1))
                        nc.scalar.dma_start(w_sb[:, tsl, :], wd[:, tsl, :])
                    w_sbs[wname] = w_sb

                def qk_pass():
                    for nb in range(NB):
                        x_sb = p1.tile([128, KT, 512], BF16, tag="x", bufs=2)
                        nchx = 4 if nb == 0 else 2
                        for tch in range(nchx):
                            tsl = slice(KT // nchx * tch, KT // nchx * (tch + 1))
                            nc.sync.dma_start(x_sb[:, tsl, :],
                                              xt_d[nb][:, tsl, :])
                        pss = [ps1.tile([128, 512], F32, tag=f"pp{i}",
                                        name=f"pp{i}")
                               for i in range(2 * HL)]
                        for t in range(KT):
                            for i, (w_sb, m) in enumerate(
                                (w_sbs[w], m) for w in ("wk", "wq")
                                for m in range(HL)
                            ):
                                nc.tensor.matmul(
                                    pss[i], w_sb[:, t, bass.ts(m, 128)],
                                    x_sb[:, t, :],
                                    start=(t == 0), stop=(t == KT - 1),
                                )
                        for i, (dst, m) in enumerate(
                            (dst, m) for dst in (kr, qr) for m in range(HL)
                        ):
                            nc.scalar.copy(dst[:, m, bass.ts(nb, 512)],
                                           pss[i])

                def v_pass():
                    w_sb = w_sbs["wv"]
                    for nb in range(NB):
                        x_sb = p1.tile([128, KT, 512], BF16, tag="x", bufs=2)
                        for tch in range(2):
                            tsl = slice(8 * tch, 8 * tch + 8)
                            nc.sync.dma_start(x_sb[:, tsl, :],
                                              xt_d[nb][:, tsl, :])
                        pss = [ps1.tile([128, 512], F32, tag=f"pp{m}",
                                        name=f"pp{m}")
                               for m in range(HL)]
                        for t in range(KT):
                            for m in range(HL):
                                nc.tensor.matmul(
                                    pss[m], x_sb[:, t, bass.ts(m, 128)],
                                    w_sb[:, t, :],
                                    start=(t == 0), stop=(t == KT - 1),
                                )
                        for m in range(HL):
                            nc.vector.tensor_copy(vv[:, nb * HL + m, :],
                                                  pss[m])
                        if nb == 0:
                            for hch in range(2):
                                nc.scalar.dma_start(
                                    wo_sb[:, 2 * hch:2 * hch + 2, :],
                                    wo_d[:, 2 * hch:2 * hch + 2, :])

                def rope(src):
                    # t = shift(src) * sin_signed; src *= cos; src += t
                    # (sign of sin folded in on host). The d-half swap is a
                    # partition shuffle - done with SBUF->SBUF DMA.
                    for h in range(HL):
                        sl = src[:, h, :]
                        tmp = p1.tile([128, N], BF16, tag="tmp", bufs=2)
                        nc.scalar.dma_start(tmp[0:64, :], sl[64:128, :])
                        nc.scalar.dma_start(tmp[64:128, :], sl[0:64, :])
                        nc.vector.tensor_tensor(tmp[:], tmp[:], sin_sb[:],
                                                op=MULT)
                        nc.vector.tensor_tensor(sl, sl, cos_sb[:], op=MULT)
                        nc.vector.tensor_tensor(sl, sl, tmp[:], op=ADD)

                qk_pass()
                rope(kr)          # DVE + DMA, overlaps v pass on PE
                rope(qr)
                v_pass()

            # ---------------- Phase 2: RoPE'd attention (bf16) ----------
            with (
                tc.tile_pool(name="p2", bufs=1) as p2,
                tc.tile_pool(name="ps_s", bufs=2, space=PSUM) as ps_s,
                tc.tile_pool(name="ps_a", bufs=2, space=PSUM) as ps_a,
            ):
                for h in range(HL):
                    for ih in range(2):
                        ihsl = bass.ts(ih, 1024)
                        a_ps = ps_a.tile([128, 1024], F32, tag="a")
                        # acc stays bf16 end-to-end: DVE runs ~4x faster in
                        # all-16-bit mode, and only 16 sequential bf16 adds
                        # happen per partition before the exact f32 matmul
                        # reduction, so the denominator error is ~1e-3.
                        acc = p2.tile([128, 1024], BF16, tag="acc", bufs=2)
                        for j in range(NT):
                            s_ps = ps_s.tile([128, 1024], F32, tag="s")
                            for f in range(2):
                                q0 = ih * 1024 + f * 512
                                nc.tensor.matmul(
                                    s_ps[:, bass.ts(f, 512)],
                                    kr[:, h, bass.ts(j, 128)],
                                    qr[:, h, q0:q0 + 512],
                                    start=True, stop=True,
                                )
                            s_exp = p2.tile([128, 1024], BF16, tag="sexp",
                                            bufs=4)
                            nc.scalar.activation(s_exp[:], s_ps[:], EXP,
                                                 scale=SCALE)
                            if j == 0:
                                nc.vector.tensor_copy(acc[:], s_exp[:])
                            else:
                                nc.vector.tensor_tensor(acc[:], acc[:],
                                                        s_exp[:], op=ADD)
                            for f in range(2):
                                nc.tensor.matmul(
                                    a_ps[:, bass.ts(f, 512)],
                                    vv[:, j, bass.ts(h, 128)],
                                    s_exp[:, bass.ts(f, 512)],
                                    start=(j == 0), stop=(j == NT - 1),
                                )
                        # softmax denominators: partition-reduce acc with a
                        # ones-column matmul (exact f32 in PSUM), fast
                        # reciprocal on VectorE, broadcast over partitions
                        # on GpSimd.
                        l_ps = ps_s.tile([1, 1024], F32, tag="s")
                        for f in range(2):
                            nc.tensor.matmul(
                                l_ps[:, bass.ts(f, 512)],
                                ones_col[:],
                                acc[:, bass.ts(f, 512)],
                                start=True, stop=True,
                            )
                        recip = p2.tile([1, 1024], F32, tag="recip", bufs=2)
                        nc.vector.reciprocal_approx_fast(recip[:], l_ps[:])
                        bc_sb = p2.tile([128, 1024], F32, tag="bcsb", bufs=2)
                        nc.gpsimd.partition_broadcast(bc_sb[:], recip[0:1, :])
                        nc.vector.tensor_tensor(ao[:, h, ihsl], a_ps[:],
                                                bc_sb[:], op=MULT)

            # ---------------- Phase 3: o_proj (bf16) --------------------
            with (
                tc.tile_pool(name="p3", bufs=1) as p3,
                tc.tile_pool(name="ps3", bufs=2, space=PSUM) as ps3,
            ):
                for m in range(NT):
                    st = p3.tile([128, D], BF16, tag="st", bufs=2)
                    for f in range(4):
                        o_ps = ps3.tile([128, 512], F32, tag=f"o{f % 2}")
                        for ct in range(HL):
                            nc.tensor.matmul(
                                o_ps[:],
                                ao[:, ct, bass.ts(m, 128)],
                                wo_sb[:, ct, bass.ts(f, 512)],
                                start=(ct == 0), stop=(ct == HL - 1),
                            )
                        if f < 3:
                            nc.scalar.copy(st[:, bass.ts(f, 512)], o_ps[:])
                        else:
                            nc.vector.tensor_copy(st[:, bass.ts(f, 512)],
                                                  o_ps[:])
                    nc.sync.dma_start(out_d[bass.ts(m, 128), :], st[:])

    nc.compile()
    return nc


def _rope_tables():
    inv_freq = 1.0 / (10000.0 ** (np.arange(0, HD, 2, dtype=np.float32) / HD))
    pos = np.arange(N, dtype=np.float32)
    freqs = pos[:, None] * inv_freq[None, :]          # [N, HD/2]
    emb = np.concatenate([freqs, freqs], axis=-1)     # [N, HD]
    cos = np.cos(emb).astype(np.float32).T.copy()     # [HD, N]
    sin = np.sin(emb).astype(np.float32).T.copy()     # [HD, N]
    sin_signed = sin.copy()
    sin_signed[0:64] *= -1.0
    return cos, sin_signed


def _make_in_maps(x, Wq, Wk, Wv, Wo):
    cos, sin_signed = _rope_tables()
    bf = ml_dtypes.bfloat16

    in_maps = []
    for c in range(N_CORES):
        b, hg = c // 4, c % 4
        cols = slice(C * hg, C * hg + C)
        xT = np.ascontiguousarray(x[b].T)                      # [D, N]
        xt = np.ascontiguousarray(
            xT.reshape(KT, 128, NB, 512).transpose(2, 1, 0, 3)
        ).astype(bf)                                           # [NB,128,KT,512]

        def wslice(W):
            wt = W[cols, :].T                                  # [D, C]
            return np.ascontiguousarray(
                wt.reshape(KT, 128, C).transpose(1, 0, 2)
            ).astype(bf)                                       # [128, KT, C]

        wo_t = Wo[:, cols].T                                   # [C, D]
        wo = np.ascontiguousarray(
            wo_t.reshape(HL, 128, D).transpose(1, 0, 2)
        ).astype(bf)                                           # [128, HL, D]

        in_maps.append({
            "xt": xt,
            "wq": wslice(Wq),
            "wk": wslice(Wk),
            "wv": wslice(Wv),
            "wo": wo,
            "cos": cos.astype(bf),
            "sin": sin_signed.astype(bf),
            "onec": np.ones((128, 1), dtype=bf),
        })
    return in_maps


def kernel(x, Wq, Wk, Wv, Wo):
    x = np.asarray(x, dtype=np.float32)
    Wq = np.asarray(Wq, dtype=np.float32)
    Wk = np.asarray(Wk, dtype=np.float32)
    Wv = np.asarray(Wv, dtype=np.float32)
    Wo = np.asarray(Wo, dtype=np.float32)

    if "nc" not in _CACHE:
        _CACHE["nc"] = _build_program()
    nc = _CACHE["nc"]

    in_maps = _make_in_maps(x, Wq, Wk, Wv, Wo)
    results = run_bass_kernel_spmd(
        nc, in_maps, core_ids=list(range(N_CORES))
    ).results

    out = np.zeros((B, N, D), dtype=np.float32)
    for c in range(N_CORES):
        out[c // 4] += np.asarray(results[c]["out"], dtype=np.float32)
    return out


# revision 13
# speedup vs baseline: 1.9673x; 1.0432x over previous
"""MHSA + RoPE kernel for Trainium2, 8 NeuronCores.

Sharding: data-parallel over batch (B=2) x tensor-parallel over heads
(16 heads -> 4 head-groups of 4). Core c handles batch c//4, heads
[4*(c%4) : 4*(c%4)+4]. Each core computes its partial o_proj output
[N, D]; host sums the 4 partials per batch (the "all-reduce").

v2 (post-trace): everything on the PE runs bf16 (HW runs f32r matmuls
~2x slower per row than bf16 despite the cost model's parity claim).
Projections run in three passes (k, q, v) with x streamed per pass and
t-outer accumulation so the first matmul starts after one DMA chunk;
RoPE on k overlaps the q pass, RoPE on q overlaps the v pass. The
attention inner loop is split per ih-half with 2-deep PSUM rings and a
3-deep exp ring so scores/exp/PV pipeline across iterations. Softmax
denominators accumulate on VectorE, partition-reduce via a ones-column
matmul, and broadcast back over partitions on GpSimd. Output is bf16.
"""

import sys

sys.path.insert(0, "/opt/trn_rl_repo")

import numpy as np
import ml_dtypes

import concourse.bass as bass
import concourse.tile as tile
from concourse import bacc, mybir
from concourse.bass_utils import run_bass_kernel_spmd

F32 = mybir.dt.float32
F32R = mybir.dt.float32r
BF16 = mybir.dt.bfloat16
MULT = mybir.AluOpType.mult
ADD = mybir.AluOpType.add
EXP = mybir.ActivationFunctionType.Exp
PSUM = bass.MemorySpace.PSUM

B, N, D = 2, 2048, 2048
H, HD = 16, 128
HL = 4            # local heads per core
C = HL * HD       # 512 local head cols
KT = D // 128     # 16 contraction tiles
NB = 4            # n-blocks of 512 for projections
NT = N // 128     # 16 j-tiles
SCALE = float(HD) ** -0.5
N_CORES = 8

_CACHE = {}


def _build_program():
    nc = bacc.Bacc("TRN2", target_bir_lowering=False, debug=False,
                   num_devices=N_CORES)

    xt_d = nc.dram_tensor("xt", [NB, 128, KT, 512], BF16, kind="ExternalInput")
    wq_d = nc.dram_tensor("wq", [128, KT, C], BF16, kind="ExternalInput")
    wk_d = nc.dram_tensor("wk", [128, KT, C], BF16, kind="ExternalInput")
    wv_d = nc.dram_tensor("wv", [128, KT, C], BF16, kind="ExternalInput")
    wo_d = nc.dram_tensor("wo", [128, HL, D], BF16, kind="ExternalInput")
    cos_d = nc.dram_tensor("cos", [128, N], BF16, kind="ExternalInput")
    sin_d = nc.dram_tensor("sin", [128, N], BF16, kind="ExternalInput")
    onec_d = nc.dram_tensor("onec", [128, 1], BF16, kind="ExternalInput")
    out_d = nc.dram_tensor("out", [N, D], BF16, kind="ExternalOutput")

    with tile.TileContext(nc) as tc:
        with tc.tile_pool(name="res", bufs=1) as res:
            qr = res.tile([128, HL, N], BF16)    # q^T per head [d, n]
            kr = res.tile([128, HL, N], BF16)    # k^T per head [d, n]
            vv = res.tile([128, NT, C], BF16)    # v natural [n, c]
            ao = res.tile([128, HL, N], BF16)    # A^T normalized [c, n]
            wo_sb = res.tile([128, HL, D], BF16)
            cos_sb = res.tile([128, N], BF16)
            sin_sb = res.tile([128, N], BF16)
            ones_col = res.tile([128, 1], BF16)
            nc.sync.dma_start(ones_col[:], onec_d[:])
            nc.sync.dma_start(cos_sb[:], cos_d[:])
            nc.sync.dma_start(sin_sb[:], sin_d[:])

            # ---------------- Phase 1: Q/K/V projections (bf16) ---------
            # q+k in one pass over x (8 psum chains), then v in a second
            # pass. x chunks stream on the SP DMA queue; weights + RoPE
            # swaps ride the Activation HWDGE queue so the two overlap.
            with (
                tc.tile_pool(name="p1", bufs=1) as p1,
                tc.tile_pool(name="ps1", bufs=1, space=PSUM) as ps1,
            ):
                w_sbs = {}
                for wd, wname in ((wk_d, "wk"), (wq_d, "wq"), (wv_d, "wv")):
                    w_sb = p1.tile([128, KT, C], BF16, tag=f"w_{wname}")
                    chunks = ([(0, 2), (2, 4), (4, 8), (8, 16)]
                              if wname == "wk" else [(0, 8), (8, 16)])
                    for t0, t1 in chunks:
                        nc.scalar.dma_start(w_sb[:, t0:t1, :], wd[:, t0:t1, :])
                    w_sbs[wname] = w_sb

                def qk_pass():
                    for nb in range(NB):
                        x_sb = p1.tile([128, KT, 512], BF16, tag="x", bufs=2)
                        chunks = ([(0, 2), (2, 4), (4, 8), (8, 16)]
                                  if nb == 0 else [(0, 8), (8, 16)])
                        for t0, t1 in chunks:
                            nc.sync.dma_start(x_sb[:, t0:t1, :],
                                              xt_d[nb][:, t0:t1, :])
                        pss = [ps1.tile([128, 512], F32, tag=f"pp{i}",
                                        name=f"pp{i}")
                               for i in range(2 * HL)]
                        for t in range(KT):
                            for i, (w_sb, m) in enumerate(
                                (w_sbs[w], m) for w in ("wk", "wq")
                                for m in range(HL)
                            ):
                                nc.tensor.matmul(
                                    pss[i], w_sb[:, t, bass.ts(m, 128)],
                                    x_sb[:, t, :],
                                    start=(t == 0), stop=(t == KT - 1),
                                )
                        for i, (dst, m) in enumerate(
                            (dst, m) for dst in (kr, qr) for m in range(HL)
                        ):
                            nc.scalar.copy(dst[:, m, bass.ts(nb, 512)],
                                           pss[i])

                def v_pass():
                    w_sb = w_sbs["wv"]
                    for nb in range(NB):
                        x_sb = p1.tile([128, KT, 512], BF16, tag="x", bufs=2)
                        for tch in range(2):
                            tsl = slice(8 * tch, 8 * tch + 8)
                            nc.sync.dma_start(x_sb[:, tsl, :],
                                              xt_d[nb][:, tsl, :])
                        pss = [ps1.tile([128, 512], F32,
                                        tag=f"pp{(nb % 2) * HL + m}",
                                        name=f"pp{m}")
                               for m in range(HL)]
                        for t in range(KT):
                            for m in range(HL):
                                nc.tensor.matmul(
                                    pss[m], x_sb[:, t, bass.ts(m, 128)],
                                    w_sb[:, t, :],
                                    start=(t == 0), stop=(t == KT - 1),
                                )
                        for m in range(HL):
                            nc.scalar.copy(vv[:, nb * HL + m, :], pss[m])
                        if nb == 0:
                            for hch in range(2):
                                nc.scalar.dma_start(
                                    wo_sb[:, 2 * hch:2 * hch + 2, :],
                                    wo_d[:, 2 * hch:2 * hch + 2, :])

                def rope(src):
                    # t = shift(src) * sin_signed; src *= cos; src += t
                    # (sign of sin folded in on host). The d-half swap is a
                    # partition shuffle - done with SBUF->SBUF DMA.
                    for h in range(HL):
                        sl = src[:, h, :]
                        tmp = p1.tile([128, N], BF16, tag="tmp", bufs=2)
                        nc.scalar.dma_start(tmp[0:64, :], sl[64:128, :])
                        nc.scalar.dma_start(tmp[64:128, :], sl[0:64, :])
                        nc.vector.tensor_tensor(tmp[:], tmp[:], sin_sb[:],
                                                op=MULT)
                        nc.vector.tensor_tensor(sl, sl, cos_sb[:], op=MULT)
                        nc.vector.tensor_tensor(sl, sl, tmp[:], op=ADD)

                qk_pass()
                rope(kr)          # DVE + DMA, overlaps v pass on PE
                rope(qr)
                v_pass()

            # ---------------- Phase 2: RoPE'd attention (bf16) ----------
            with (
                tc.tile_pool(name="p2", bufs=1) as p2,
                tc.tile_pool(name="ps_s", bufs=2, space=PSUM) as ps_s,
                tc.tile_pool(name="ps_a", bufs=2, space=PSUM) as ps_a,
            ):
                for h in range(HL):
                    for ih in range(2):
                        ihsl = bass.ts(ih, 1024)
                        a_ps = ps_a.tile([128, 1024], F32, tag="a")
                        # acc stays bf16 end-to-end: only 16 sequential bf16
                        # adds happen per partition before the exact f32
                        # matmul reduction, so the denominator error ~1e-3.
                        acc = p2.tile([128, 1024], BF16, tag="acc", bufs=2)

                        # scores run one j ahead of PV so the in-order PE
                        # queue never waits on exp(j): queue order is
                        # S(0) S(1) P(0) S(2) P(1) ... P(15).
                        def scores(j):
                            s_ps = ps_s.tile([128, 1024], F32, tag="s",
                                             name="s_ps")
                            for f in range(2):
                                q0 = ih * 1024 + f * 512
                                nc.tensor.matmul(
                                    s_ps[:, bass.ts(f, 512)],
                                    kr[:, h, bass.ts(j, 128)],
                                    qr[:, h, q0:q0 + 512],
                                    start=True, stop=True,
                                )
                            s_exp = p2.tile([128, 1024], BF16, tag="sexp",
                                            bufs=4, name="s_exp")
                            nc.scalar.activation(s_exp[:], s_ps[:], EXP,
                                                 scale=SCALE)
                            return s_exp

                        s_exp_next = scores(0)
                        for j in range(NT):
                            s_exp = s_exp_next
                            if j + 1 < NT:
                                s_exp_next = scores(j + 1)
                            if j == 0:
                                nc.gpsimd.tensor_copy(acc[:], s_exp[:])
                            else:
                                nc.vector.tensor_tensor(acc[:], acc[:],
                                                        s_exp[:], op=ADD)
                            for f in range(2):
                                nc.tensor.matmul(
                                    a_ps[:, bass.ts(f, 512)],
                                    vv[:, j, bass.ts(h, 128)],
                                    s_exp[:, bass.ts(f, 512)],
                                    start=(j == 0), stop=(j == NT - 1),
                                )
                        # softmax denominators: partition-reduce acc with a
                        # ones-column matmul (exact f32 in PSUM), fast
                        # reciprocal on VectorE, broadcast over partitions
                        # and apply on GpSimd.
                        l_ps = ps_s.tile([1, 1024], F32, tag="s")
                        for f in range(2):
                            nc.tensor.matmul(
                                l_ps[:, bass.ts(f, 512)],
                                ones_col[:],
                                acc[:, bass.ts(f, 512)],
                                start=True, stop=True,
                            )
                        recip = p2.tile([1, 1024], F32, tag="recip", bufs=2)
                        nc.vector.reciprocal_approx_fast(recip[:], l_ps[:])
                        bc_sb = p2.tile([128, 1024], F32, tag="bcsb", bufs=2)
                        nc.gpsimd.partition_broadcast(bc_sb[:], recip[0:1, :])
                        nc.vector.tensor_tensor(ao[:, h, ihsl], a_ps[:],
                                                bc_sb[:], op=MULT)

            # ---------------- Phase 3: o_proj (bf16) --------------------
            with (
                tc.tile_pool(name="p3", bufs=1) as p3,
                tc.tile_pool(name="ps3", bufs=2, space=PSUM) as ps3,
            ):
                for m in range(NT):
                    st = p3.tile([128, D], BF16, tag="st", bufs=2)
                    for f in range(4):
                        o_ps = ps3.tile([128, 512], F32, tag=f"o{f % 2}")
                        for ct in range(HL):
                            nc.tensor.matmul(
                                o_ps[:],
                                ao[:, ct, bass.ts(m, 128)],
                                wo_sb[:, ct, bass.ts(f, 512)],
                                start=(ct == 0), stop=(ct == HL - 1),
                            )
                        if f < 3:
                            nc.scalar.copy(st[:, bass.ts(f, 512)], o_ps[:])
                        else:
                            nc.vector.tensor_copy(st[:, bass.ts(f, 512)],
                                                  o_ps[:])
                    nc.sync.dma_start(out_d[bass.ts(m, 128), :], st[:])

    nc.compile()
    return nc


def _rope_tables():
    inv_freq = 1.0 / (10000.0 ** (np.arange(0, HD, 2, dtype=np.float32) / HD))
    pos = np.arange(N, dtype=np.float32)
    freqs = pos[:, None] * inv_freq[None, :]          # [N, HD/2]
    emb = np.concatenate([freqs, freqs], axis=-1)     # [N, HD]
    cos = np.cos(emb).astype(np.float32).T.copy()     # [HD, N]
    sin = np.sin(emb).astype(np.float32).T.copy()     # [HD, N]
    sin_signed = sin.copy()
    sin_signed[0:64] *= -1.0
    return cos, sin_signed


def _make_in_maps(x, Wq, Wk, Wv, Wo):
    cos, sin_signed = _rope_tables()
    bf = ml_dtypes.bfloat16

    in_maps = []
    for c in range(N_CORES):
        b, hg = c // 4, c % 4
        cols = slice(C * hg, C * hg + C)
        xT = np.ascontiguousarray(x[b].T)                      # [D, N]
        xt = np.ascontiguousarray(
            xT.reshape(KT, 128, NB, 512).transpose(2, 1, 0, 3)
        ).astype(bf)                                           # [NB,128,KT,512]

        def wslice(W):
            wt = W[cols, :].T                                  # [D, C]
            return np.ascontiguousarray(
                wt.reshape(KT, 128, C).transpose(1, 0, 2)
            ).astype(bf)                                       # [128, KT, C]

        wo_t = Wo[:, cols].T                                   # [C, D]
        wo = np.ascontiguousarray(
            wo_t.reshape(HL, 128, D).transpose(1, 0, 2)
        ).astype(bf)                                           # [128, HL, D]

        in_maps.append({
            "xt": xt,
            "wq": wslice(Wq),
            "wk": wslice(Wk),
            "wv": wslice(Wv),
            "wo": wo,
            "cos": cos.astype(bf),
            "sin": sin_signed.astype(bf),
            "onec": np.ones((128, 1), dtype=bf),
        })
    return in_maps


def kernel(x, Wq, Wk, Wv, Wo):
    x = np.asarray(x, dtype=np.float32)
    Wq = np.asarray(Wq, dtype=np.float32)
    Wk = np.asarray(Wk, dtype=np.float32)
    Wv = np.asarray(Wv, dtype=np.float32)
    Wo = np.asarray(Wo, dtype=np.float32)

    if "nc" not in _CACHE:
        _CACHE["nc"] = _build_program()
    nc = _CACHE["nc"]

    in_maps = _make_in_maps(x, Wq, Wk, Wv, Wo)
    results = run_bass_kernel_spmd(
        nc, in_maps, core_ids=list(range(N_CORES))
    ).results

    out = np.zeros((B, N, D), dtype=np.float32)
    for c in range(N_CORES):
        out[c // 4] += np.asarray(results[c]["out"], dtype=np.float32)
    return out


# revision 15
# speedup vs baseline: 2.1109x; 1.0730x over previous
"""MHSA + RoPE kernel for Trainium2, 8 NeuronCores.

Sharding: data-parallel over batch (B=2) x tensor-parallel over heads
(16 heads -> 4 head-groups of 4). Core c handles batch c//4, heads
[4*(c%4) : 4*(c%4)+4]. Each core computes its partial o_proj output
[N, D]; host sums the 4 partials per batch (the "all-reduce").

v2 (post-trace): everything on the PE runs bf16 (HW runs f32r matmuls
~2x slower per row than bf16 despite the cost model's parity claim).
Projections run in three passes (k, q, v) with x streamed per pass and
t-outer accumulation so the first matmul starts after one DMA chunk;
RoPE on k overlaps the q pass, RoPE on q overlaps the v pass. The
attention inner loop is split per ih-half with 2-deep PSUM rings and a
3-deep exp ring so scores/exp/PV pipeline across iterations. Softmax
denominators accumulate on VectorE, partition-reduce via a ones-column
matmul, and broadcast back over partitions on GpSimd. Output is bf16.
"""

import sys

sys.path.insert(0, "/opt/trn_rl_repo")

import numpy as np
import ml_dtypes

import concourse.bass as bass
import concourse.tile as tile
from concourse import bacc, mybir
from concourse.bass_utils import run_bass_kernel_spmd

F32 = mybir.dt.float32
F32R = mybir.dt.float32r
BF16 = mybir.dt.bfloat16
MULT = mybir.AluOpType.mult
ADD = mybir.AluOpType.add
EXP = mybir.ActivationFunctionType.Exp
PSUM = bass.MemorySpace.PSUM

B, N, D = 2, 2048, 2048
H, HD = 16, 128
HL = 4            # local heads per core
C = HL * HD       # 512 local head cols
KT = D // 128     # 16 contraction tiles
NB = 4            # n-blocks of 512 for projections
NT = N // 128     # 16 j-tiles
SCALE = float(HD) ** -0.5
N_CORES = 8

_CACHE = {}


def _build_program():
    nc = bacc.Bacc("TRN2", target_bir_lowering=False, debug=False,
                   num_devices=N_CORES)

    xt_d = nc.dram_tensor("xt", [NB, 128, KT, 512], BF16, kind="ExternalInput")
    wq_d = nc.dram_tensor("wq", [128, KT, C], BF16, kind="ExternalInput")
    wk_d = nc.dram_tensor("wk", [128, KT, C], BF16, kind="ExternalInput")
    wv_d = nc.dram_tensor("wv", [128, KT, C], BF16, kind="ExternalInput")
    wo_d = nc.dram_tensor("wo", [128, HL, D], BF16, kind="ExternalInput")
    cos_d = nc.dram_tensor("cos", [128, N], BF16, kind="ExternalInput")
    sin_d = nc.dram_tensor("sin", [128, N], BF16, kind="ExternalInput")
    onec_d = nc.dram_tensor("onec", [128, 1], BF16, kind="ExternalInput")
    out_d = nc.dram_tensor("out", [N, D], BF16, kind="ExternalOutput")

    with tile.TileContext(nc) as tc:
        with tc.tile_pool(name="res", bufs=1) as res:
            qr = res.tile([128, HL, N], BF16)    # q^T per head [d, n]
            kr = res.tile([128, HL, N], BF16)    # k^T per head [d, n]
            vv = res.tile([128, NT, C], BF16)    # v natural [n, c]
            ao = res.tile([128, HL, N], BF16)    # A^T normalized [c, n]
            wo_sb = res.tile([128, HL, D], BF16)
            cos_sb = res.tile([128, N], BF16)
            sin_sb = res.tile([128, N], BF16)
            ones_col = res.tile([128, 1], BF16)
            nc.sync.dma_start(ones_col[:], onec_d[:])
            nc.sync.dma_start(cos_sb[:], cos_d[:])
            nc.sync.dma_start(sin_sb[:], sin_d[:])

            # ---------------- Phase 1: Q/K/V projections (bf16) ---------
            # q+k in one pass over x (8 psum chains), then v in a second
            # pass. x chunks stream on the SP DMA queue; weights + RoPE
            # swaps ride the Activation HWDGE queue so the two overlap.
            with (
                tc.tile_pool(name="p1", bufs=1) as p1,
                tc.tile_pool(name="ps1", bufs=1, space=PSUM) as ps1,
            ):
                w_sbs = {}
                for wd, wname in ((wk_d, "wk"), (wq_d, "wq"), (wv_d, "wv")):
                    w_sb = p1.tile([128, KT, C], BF16, tag=f"w_{wname}")
                    chunks = ([(0, 2), (2, 4), (4, 8), (8, 16)]
                              if wname == "wk" else [(0, 8), (8, 16)])
                    for t0, t1 in chunks:
                        nc.scalar.dma_start(w_sb[:, t0:t1, :], wd[:, t0:t1, :])
                    w_sbs[wname] = w_sb

                def qk_pass():
                    for nb in range(NB):
                        x_sb = p1.tile([128, KT, 512], BF16, tag="x", bufs=2)
                        chunks = ([(0, 2), (2, 4), (4, 8), (8, 16)]
                                  if nb == 0 else [(0, 8), (8, 16)])
                        for t0, t1 in chunks:
                            nc.sync.dma_start(x_sb[:, t0:t1, :],
                                              xt_d[nb][:, t0:t1, :])
                        pss = [ps1.tile([128, 512], F32, tag=f"pp{i}",
                                        name=f"pp{i}")
                               for i in range(2 * HL)]
                        for t in range(KT):
                            for i, (w_sb, m) in enumerate(
                                (w_sbs[w], m) for w in ("wk", "wq")
                                for m in range(HL)
                            ):
                                nc.tensor.matmul(
                                    pss[i], w_sb[:, t, bass.ts(m, 128)],
                                    x_sb[:, t, :],
                                    start=(t == 0), stop=(t == KT - 1),
                                )
                        for i, (dst, m) in enumerate(
                            (dst, m) for dst in (kr, qr) for m in range(HL)
                        ):
                            nc.scalar.copy(dst[:, m, bass.ts(nb, 512)],
                                           pss[i])

                def v_pass():
                    w_sb = w_sbs["wv"]
                    for nb in range(NB):
                        x_sb = p1.tile([128, KT, 512], BF16, tag="x", bufs=2)
                        for tch in range(2):
                            tsl = slice(8 * tch, 8 * tch + 8)
                            nc.sync.dma_start(x_sb[:, tsl, :],
                                              xt_d[nb][:, tsl, :])
                        pss = [ps1.tile([128, 512], F32,
                                        tag=f"pp{(nb % 2) * HL + m}",
                                        name=f"pp{m}")
                               for m in range(HL)]
                        for t in range(KT):
                            for m in range(HL):
                                nc.tensor.matmul(
                                    pss[m], x_sb[:, t, bass.ts(m, 128)],
                                    w_sb[:, t, :],
                                    start=(t == 0), stop=(t == KT - 1),
                                )
                        for m in range(HL):
                            nc.scalar.copy(vv[:, nb * HL + m, :], pss[m])
                        if nb == 0:
                            for hch in range(2):
                                nc.scalar.dma_start(
                                    wo_sb[:, 2 * hch:2 * hch + 2, :],
                                    wo_d[:, 2 * hch:2 * hch + 2, :])

                def rope(src):
                    # t = shift(src) * sin_signed; src *= cos; src += t
                    # (sign of sin folded in on host). The d-half swap is a
                    # partition shuffle - done with SBUF->SBUF DMA.
                    for h in range(HL):
                        sl = src[:, h, :]
                        tmp = p1.tile([128, N], BF16, tag="tmp", bufs=2)
                        nc.scalar.dma_start(tmp[0:64, :], sl[64:128, :])
                        nc.scalar.dma_start(tmp[64:128, :], sl[0:64, :])
                        nc.vector.tensor_tensor(tmp[:], tmp[:], sin_sb[:],
                                                op=MULT)
                        nc.vector.tensor_tensor(sl, sl, cos_sb[:], op=MULT)
                        nc.vector.tensor_tensor(sl, sl, tmp[:], op=ADD)

                qk_pass()
                rope(kr)          # DVE + DMA, overlaps v pass on PE
                rope(qr)
                v_pass()

            # ---------------- Phase 2: RoPE'd attention (bf16) ----------
            with (
                tc.tile_pool(name="p2", bufs=1) as p2,
                tc.tile_pool(name="ps_s", bufs=2, space=PSUM) as ps_s,
                tc.tile_pool(name="ps_a", bufs=2, space=PSUM) as ps_a,
            ):
                # One software-pipelined stream over all (head, ih-half,
                # j) steps: the scores/exp stream runs LOOKAHEAD steps
                # ahead of the PV/accumulate stream, across window
                # boundaries, so neither PE nor ActE ever re-syncs at a
                # window edge. acc stays bf16 end-to-end (only 16
                # sequential bf16 adds per partition before the exact f32
                # matmul reduction -> denominator error ~1e-3).
                windows = [(h, ih) for h in range(HL) for ih in range(2)]
                seq = [(w, j) for w in range(len(windows)) for j in range(NT)]
                LOOKAHEAD = 2
                state = {}

                def emit_scores(w, j):
                    h, ih = windows[w]
                    s_ps = ps_s.tile([128, 1024], F32, tag="s", name="s_ps")
                    for f in range(2):
                        q0 = ih * 1024 + f * 512
                        nc.tensor.matmul(
                            s_ps[:, bass.ts(f, 512)],
                            kr[:, h, bass.ts(j, 128)],
                            qr[:, h, q0:q0 + 512],
                            start=True, stop=True,
                        )
                    s_exp = p2.tile([128, 1024], BF16, tag="sexp",
                                    bufs=6, name="s_exp")
                    nc.scalar.activation(s_exp[:], s_ps[:], EXP, scale=SCALE)
                    state[(w, j)] = s_exp

                def emit_consume(w, j):
                    h, ih = windows[w]
                    if j == 0:
                        state[w, "a"] = ps_a.tile([128, 1024], F32, tag="a",
                                                  name="a_ps")
                        acc = p2.tile([128, 1024], BF16, tag="acc", bufs=2,
                                      name="acc")
                        nc.gpsimd.memset(acc[:], 0)
                        state[w, "acc"] = acc
                    a_ps, acc = state[w, "a"], state[w, "acc"]
                    s_exp = state.pop((w, j))
                    nc.vector.tensor_tensor(acc[:], acc[:], s_exp[:], op=ADD)
                    for f in range(2):
                        nc.tensor.matmul(
                            a_ps[:, bass.ts(f, 512)],
                            vv[:, j, bass.ts(h, 128)],
                            s_exp[:, bass.ts(f, 512)],
                            start=(j == 0), stop=(j == NT - 1),
                        )
                    if j == NT - 1:
                        # softmax denominators: partition-reduce acc with a
                        # ones-column matmul (exact f32 in PSUM), fast
                        # reciprocal on VectorE, broadcast over partitions
                        # on GpSimd, apply on VectorE.
                        l_ps = ps_s.tile([1, 1024], F32, tag="s")
                        for f in range(2):
                            nc.tensor.matmul(
                                l_ps[:, bass.ts(f, 512)],
                                ones_col[:],
                                acc[:, bass.ts(f, 512)],
                                start=True, stop=True,
                            )
                        recip = p2.tile([1, 1024], F32, tag="recip", bufs=2)
                        nc.vector.reciprocal_approx_fast(recip[:], l_ps[:])
                        bc_sb = p2.tile([128, 1024], F32, tag="bcsb", bufs=2)
                        nc.gpsimd.partition_broadcast(bc_sb[:],
                                                      recip[0:1, :])
                        nc.vector.tensor_tensor(ao[:, h, bass.ts(ih, 1024)],
                                                a_ps[:], bc_sb[:], op=MULT)

                for idx in range(len(seq) + LOOKAHEAD):
                    if idx < len(seq):
                        emit_scores(*seq[idx])
                    if idx >= LOOKAHEAD:
                        emit_consume(*seq[idx - LOOKAHEAD])

            # ---------------- Phase 3: o_proj (bf16) --------------------
            with (
                tc.tile_pool(name="p3", bufs=1) as p3,
                tc.tile_pool(name="ps3", bufs=2, space=PSUM) as ps3,
            ):
                for m in range(NT):
                    st = p3.tile([128, D], BF16, tag="st", bufs=2)
                    for f in range(4):
                        o_ps = ps3.tile([128, 512], F32, tag=f"o{f % 2}")
                        for ct in range(HL):
                            nc.tensor.matmul(
                                o_ps[:],
                                ao[:, ct, bass.ts(m, 128)],
                                wo_sb[:, ct, bass.ts(f, 512)],
                                start=(ct == 0), stop=(ct == HL - 1),
                            )
                        if f < 3:
                            nc.scalar.copy(st[:, bass.ts(f, 512)], o_ps[:])
                        else:
                            nc.vector.tensor_copy(st[:, bass.ts(f, 512)],
                                                  o_ps[:])
                    nc.sync.dma_start(out_d[bass.ts(m, 128), :], st[:])

    nc.compile()
    return nc


def _rope_tables():
    inv_freq = 1.0 / (10000.0 ** (np.arange(0, HD, 2, dtype=np.float32) / HD))
    pos = np.arange(N, dtype=np.float32)
    freqs = pos[:, None] * inv_freq[None, :]          # [N, HD/2]
    emb = np.concatenate([freqs, freqs], axis=-1)     # [N, HD]
    cos = np.cos(emb).astype(np.float32).T.copy()     # [HD, N]
    sin = np.sin(emb).astype(np.float32).T.copy()     # [HD, N]
    sin_signed = sin.copy()
    sin_signed[0:64] *= -1.0
    return cos, sin_signed


def _make_in_maps(x, Wq, Wk, Wv, Wo):
    cos, sin_signed = _rope_tables()
    bf = ml_dtypes.bfloat16

    in_maps = []
    for c in range(N_CORES):
        b, hg = c // 4, c % 4
        cols = slice(C * hg, C * hg + C)
        xT = np.ascontiguousarray(x[b].T)                      # [D, N]
        xt = np.ascontiguousarray(
            xT.reshape(KT, 128, NB, 512).transpose(2, 1, 0, 3)
        ).astype(bf)                                           # [NB,128,KT,512]

        def wslice(W):
            wt = W[cols, :].T                                  # [D, C]
            return np.ascontiguousarray(
                wt.reshape(KT, 128, C).transpose(1, 0, 2)
            ).astype(bf)                                       # [128, KT, C]

        wo_t = Wo[:, cols].T                                   # [C, D]
        wo = np.ascontiguousarray(
            wo_t.reshape(HL, 128, D).transpose(1, 0, 2)
        ).astype(bf)                                           # [128, HL, D]

        in_maps.append({
            "xt": xt,
            "wq": wslice(Wq),
            "wk": wslice(Wk),
            "wv": wslice(Wv),
            "wo": wo,
            "cos": cos.astype(bf),
            "sin": sin_signed.astype(bf),
            "onec": np.ones((128, 1), dtype=bf),
        })
    return in_maps


def kernel(x, Wq, Wk, Wv, Wo):
    x = np.asarray(x, dtype=np.float32)
    Wq = np.asarray(Wq, dtype=np.float32)
    Wk = np.asarray(Wk, dtype=np.float32)
    Wv = np.asarray(Wv, dtype=np.float32)
    Wo = np.asarray(Wo, dtype=np.float32)

    if "nc" not in _CACHE:
        _CACHE["nc"] = _build_program()
    nc = _CACHE["nc"]

    in_maps = _make_in_maps(x, Wq, Wk, Wv, Wo)
    results = run_bass_kernel_spmd(
        nc, in_maps, core_ids=list(range(N_CORES))
    ).results

    out = np.zeros((B, N, D), dtype=np.float32)
    for c in range(N_CORES):
        out[c // 4] += np.asarray(results[c]["out"], dtype=np.float32)
    return out
